# revision 29
# baseline (speedup 1.0000x reference)
import sys
sys.path.insert(0, '/opt/trn_rl_repo')
import numpy as np
import ml_dtypes
import jax
import jax.numpy as jnp
from jax.sharding import Mesh, PartitionSpec as P, NamedSharding
from jax.experimental.shard_map import shard_map

import concourse.bass as bass
import concourse.mybir as mybir
from concourse.bass2jax import (
    _bass_exec_p, install_neuronx_cc_hook, partition_id_tensor)

# Problem: y[b,s,o] = x[b]@W.T + bias + (x[b]@a[idx[b]].T)@b[idx[b]].T
# B=8 batch elements -> data-parallel, one per NeuronCore.
#
# The axon relay serializes all transfers at ~50MB/s up / ~41MB/s down with
# no up/down overlap, so wall time is dominated by tunnel bytes. The runner
# minimizes them: x uploads int8 (per-512-block abs-max scales) sharded by
# batch, W uploads int8 *sharded* by rows (2MB/core) and is dequantized +
# all-gathered on device, y downloads int8 (per-512-block scales) and is
# dequantized to f32 on the host. Quantization block scales keep the max
# rel err ~8.5e-3 against the f32 reference (gate 2e-2).
B, S, D, RANK = 8, 2048, 4096, 16
P128 = 128
KT = D // P128       # 32 contraction tiles
NQ = 4               # s-quarters
SQ = S // NQ         # 512
NJ = 8               # o-blocks of 512
OJ = D // NJ         # 512
NT = SQ // P128      # 4 s-tiles per quarter
NGROUP = NQ * NJ * NT  # 128 output groups of [128 s, 512 o]

_BF = mybir.dt.bfloat16
_F32 = mybir.dt.float32


def build_nc():
    nc = bass.Bass()
    xt = nc.declare_dram_parameter("xt", [D, S], _BF, isOutput=False)
    wt = nc.declare_dram_parameter("wt", [D, D], _BF, isOutput=False)
    at = nc.declare_dram_parameter("at", [D, RANK], _BF, isOutput=False)
    bt = nc.declare_dram_parameter("bt", [RANK + 1, D], _BF, isOutput=False)
    ones = nc.declare_dram_parameter("ones", [1, S], _BF, isOutput=False)
    y = nc.declare_dram_parameter("y", [S, D], _F32, isOutput=True)

    xt_t = xt.rearrange("(k p) s -> p k s", p=P128)
    wt_t = wt.rearrange("(k p) o -> p k o", p=P128)
    at_t = at.rearrange("(k p) r -> p k r", p=P128)

    with (
        nc.sbuf_tensor([P128, 2, KT, SQ], _BF) as x_sb,
        nc.sbuf_tensor([P128, 2, KT, OJ], _BF) as w_sb,
        nc.sbuf_tensor([P128, KT, RANK], _BF) as at_sb,
        nc.sbuf_tensor([RANK + 1, D], _BF) as bt_sb,
        nc.sbuf_tensor([RANK + 1, S], _BF) as inter_sb,
        nc.sbuf_tensor([P128, 4, OJ], _F32) as out_sb,
        nc.psum_tensor([P128, 7, OJ], _F32) as psum_y,
        nc.psum_tensor([P128, SQ], _F32) as psum_i,
        nc.semaphore("x_sem") as x_sem,
        nc.semaphore("w_sem") as w_sem,
        nc.semaphore("c_sem") as c_sem,
        nc.semaphore("pe_sem") as pe_sem,
        nc.semaphore("pei_sem") as pei_sem,
        nc.semaphore("dve_sem") as dve_sem,
        nc.semaphore("ev_sem") as ev_sem,
        nc.semaphore("st_sem") as st_sem,
        nc.Block() as block,
    ):
        @block.sync
        def _(sync):
            sync.dma_start(at_sb[:], at_t).then_inc(c_sem, 16)
            sync.dma_start(bt_sb[:], bt[:, :]).then_inc(c_sem, 16)
            sync.dma_start(inter_sb[RANK:RANK + 1, :], ones[:, :]).then_inc(c_sem, 16)
            for q in range(NQ):
                if q >= 2:
                    sync.wait_ge(ev_sem, NJ * NT * (q - 1))
                sync.dma_start(
                    x_sb[:, q % 2], xt_t[:, :, q * SQ:(q + 1) * SQ]
                ).then_inc(x_sem, 16)
                for j in range(NJ):
                    wj = q * NJ + j
                    if wj >= 2:
                        sync.wait_ge(ev_sem, NT * (wj - 1))
                    sync.dma_start(
                        w_sb[:, j % 2], wt_t[:, :, j * OJ:(j + 1) * OJ]
                    ).then_inc(w_sem, 16)

        @block.tensor
        def _(tensor):
            tensor.wait_ge(c_sem, 48)
            g = 0
            for q in range(NQ):
                tensor.wait_ge(x_sem, 16 * (q + 1))
                if q > 0:
                    tensor.wait_ge(dve_sem, q)     # psum_i WAR
                for i in range(KT):
                    mm = nc.tensor.matmul(
                        psum_i[0:RANK, :], at_sb[:, i, :], x_sb[:, q % 2, i, :],
                        start=(i == 0), stop=(i == KT - 1),
                    )
                mm.then_inc(pei_sem, 1)
                for j in range(NJ):
                    wj = q * NJ + j
                    tensor.wait_ge(w_sem, 16 * (wj + 1))
                    for t in range(NT):
                        st = q * NT + t
                        if g >= 7:
                            tensor.wait_ge(ev_sem, g - 6)
                        for i in range(KT):
                            nc.tensor.matmul(
                                psum_y[:, g % 7, :],
                                x_sb[:, q % 2, i, t * P128:(t + 1) * P128],
                                w_sb[:, j % 2, i, :],
                                start=(i == 0), stop=False,
                            )
                        tensor.wait_ge(dve_sem, q + 1)
                        nc.tensor.matmul(
                            psum_y[:, g % 7, :],
                            inter_sb[:, st * P128:(st + 1) * P128],
                            bt_sb[:, j * OJ:(j + 1) * OJ],
                            start=False, stop=True,
                        ).then_inc(pe_sem, 1)
                        g += 1

        @block.vector
        def _(vector):
            for q in range(NQ):
                vector.wait_ge(pei_sem, q + 1)
                nc.vector.tensor_copy(
                    inter_sb[0:RANK, q * SQ:(q + 1) * SQ], psum_i[0:RANK, :]
                ).then_inc(dve_sem, 1)

        @block.scalar
        def _(scalar):
            for g in range(NGROUP):
                scalar.wait_ge(pe_sem, g + 1)
                if g >= 4:
                    scalar.wait_ge(st_sem, 16 * (g - 3))
                nc.scalar.copy(out_sb[:, g % 4, :], psum_y[:, g % 7, :]).then_inc(
                    ev_sem, 1
                )

        @block.gpsimd
        def _(gpsimd):
            for g in range(NGROUP):
                q, rem = divmod(g, NJ * NT)
                j, t = divmod(rem, NT)
                st = q * NT + t
                gpsimd.wait_ge(ev_sem, g + 1)
                gpsimd.dma_start(
                    y[st * P128:(st + 1) * P128, j * OJ:(j + 1) * OJ], out_sb[:, g % 4, :]
                ).then_inc(st_sem, 16)

    return nc


_STATE = {}


def _get_state():
    if _STATE:
        return _STATE
    install_neuronx_cc_hook()
    nc = build_nc()
    devs = jax.devices()[:B]
    mesh = Mesh(np.array(devs), ("core",))

    sh_x = NamedSharding(mesh, P("core", None, None))
    sh_w = NamedSharding(mesh, P("core", None))
    sh_r2 = NamedSharding(mesh, P("core", None))

    # stage pre_w: dequantize the row-sharded int8 W, all-gather, transpose.
    # Output is per-core stacked (P("core") over [8*4096, 4096]) so every bass
    # operand uses P("core") like run_bass_via_pjrt does -- replicated
    # operands to the bass custom-call crash the worker.
    def pre_w_body(wq_sh, wsc_sh):              # [512, 4096] i8, [512, 8] f32
        wf = wq_sh.reshape(512, 8, 512).astype(jnp.float32) * wsc_sh[:, :, None]
        wbf = wf.reshape(512, D).astype(jnp.bfloat16)
        w_full = jax.lax.all_gather(wbf, "core", axis=0, tiled=True)  # [4096,4096]
        return w_full.T                          # wt = W.T  [D_in, D_out]

    pre_w = jax.jit(shard_map(
        pre_w_body, mesh=mesh, in_specs=(P("core", None), P("core", None)),
        out_specs=P("core", None), check_rep=False))

    # stage pre_x: per-core dequantize int8 x (per-512-block scales) + transpose
    def pre_x_body(xi_sh, xs_sh):               # [1,2048,4096] i8, [1,2048,8] f32
        xf = xi_sh[0].reshape(S, 8, 512).astype(jnp.float32) * xs_sh[0][:, :, None]
        return xf.reshape(S, D).astype(jnp.bfloat16).T   # [4096, 2048]

    pre_x = jax.jit(shard_map(
        pre_x_body, mesh=mesh,
        in_specs=(P("core", None, None), P("core", None, None)),
        out_specs=P("core", None), check_rep=False))

    # bass stage: the hand-written kernel, one batch element per core.
    # The trailing partition_id operand is mandatory: bass.Bass() declares a
    # partition_id ExternalInput, and leaving it unbound crashes the worker.
    out_avals = (jax.core.ShapedArray((S, D), np.float32),)

    def bass_body(xt, wt, at, bt, ones):
        outs = _bass_exec_p.bind(
            xt, wt, at, bt, ones, partition_id_tensor(),
            out_avals=out_avals,
            in_names=("xt", "wt", "at", "bt", "ones", "partition_id"),
            out_names=("y",),
            lowering_input_output_aliases=(),
            sim_require_finite=True,
            sim_require_nnan=True,
            nc=nc,
        )
        return outs[0]

    bass_sm = jax.jit(shard_map(
        bass_body, mesh=mesh,
        in_specs=(P("core", None), P("core", None), P("core", None),
                  P("core", None), P("core")),
        out_specs=P("core", None), check_rep=False), keep_unused=True)

    # stage post: quantize y to int8 with per-512-block scales for the download
    def post_body(y):                           # [2048, 4096] f32 per core
        yr = y.reshape(S, 8, 512)
        m = jnp.max(jnp.abs(yr), axis=2, keepdims=True)
        scale = jnp.maximum(m, 1e-30) * (1.0 / 127.0)
        yi = jnp.round(yr / scale).astype(jnp.int8).reshape(S, D)
        return yi, scale[:, :, 0]

    post = jax.jit(shard_map(
        post_body, mesh=mesh, in_specs=P("core", None),
        out_specs=(P("core", None), P("core", None)), check_rep=False))

    _STATE.update(dict(nc=nc, mesh=mesh, sh_x=sh_x, sh_w=sh_w, sh_r2=sh_r2,
                       pre_w=pre_w, pre_x=pre_x, bass_sm=bass_sm, post=post))
    return _STATE


def _run(x, W, bias, lora_a, lora_b, adapter_indices):
    st = _get_state()
    mesh = st["mesh"]

    # small tables on host (bf16, a few MB total)
    idx = [int(i) for i in np.asarray(adapter_indices)]
    mw = np.abs(W).reshape(D, 8, 512).max(axis=-1)              # [4096, 8]
    wsc = (np.maximum(mw, 1e-30) * np.float32(1.0 / 127.0)).astype(np.float32)
    wq = np.rint(W.reshape(D, 8, 512) * (np.float32(1.0) / wsc)[:, :, None]
                 ).astype(np.int8).reshape(D, D)
    at_g = np.concatenate(
        [lora_a[i].T for i in idx], axis=0).astype(ml_dtypes.bfloat16)   # [8*4096, 16]
    bias32 = bias.astype(np.float32)
    bt_g = np.concatenate(
        [np.concatenate([lora_b[i].astype(np.float32).T, bias32[None, :]], axis=0)
         for i in idx], axis=0).astype(ml_dtypes.bfloat16)      # [8*17, 4096]
    ones_g = np.ones((B, S), dtype=ml_dtypes.bfloat16)

    # dispatch W + tables first so the relay streams them while we quantize x
    wd = jax.device_put(wq, st["sh_w"])
    wscd = jax.device_put(wsc, NamedSharding(mesh, P("core")))
    atd = jax.device_put(at_g, st["sh_r2"])
    btd = jax.device_put(bt_g, st["sh_r2"])
    onesd = jax.device_put(ones_g, NamedSharding(mesh, P("core")))
    wtd = st["pre_w"](wd, wscd)

    # quantize x to int8 with per-row abs-max scales (halves the upload).
    # Fused passes with preallocated buffers; round-half-up via the
    # +128.5-truncate-xor trick (uint8 cast truncates, values positive).
    tmp = _STATE.get("tmp_f32")
    if tmp is None:
        tmp = np.empty((B, S, D), np.float32)
        _STATE["tmp_f32"] = tmp
        _STATE["u8"] = np.empty((B, S, D), np.uint8)
        _STATE["mxb"] = np.empty((B, S, 8), np.float32)
    u8 = _STATE["u8"]
    np.abs(x, out=tmp)
    mx = np.max(tmp.reshape(B, S, 8, 512), axis=-1, out=_STATE["mxb"])
    inv = np.float32(127.0) / np.maximum(mx, 1e-30, out=mx)
    np.multiply(x.reshape(B, S, 8, 512), inv[:, :, :, None],
                out=tmp.reshape(B, S, 8, 512))
    tmp += np.float32(128.5)
    np.copyto(u8, tmp, casting='unsafe')
    np.bitwise_xor(u8, 0x80, out=u8)
    xi8 = u8.view(np.int8)
    xs = (np.float32(1.0) / inv).astype(np.float32)                # [8,2048,8]
    xd = jax.device_put(xi8, st["sh_x"])
    xsd = jax.device_put(xs, NamedSharding(mesh, P("core", None, None)))
    xtd = st["pre_x"](xd, xsd)

    yd = st["bass_sm"](xtd, wtd, atd, btd, onesd)
    yi, ysc = st["post"](yd)
    yi_h, ysc_h = jax.device_get((yi, ysc))     # [16384,4096] i8, [16384,8] f32
    # free device buffers eagerly so they don't pile up on the terminal
    for a in (xd, xsd, xtd, wd, wscd, wtd, atd, btd, onesd, yd, yi, ysc):
        try:
            a.delete()
        except Exception:
            pass
    # double-buffered output so two successive calls don't alias one array
    bufs = _STATE.get("out_bufs")
    if bufs is None:
        bufs = [np.empty((B * S, D), dtype=np.float32) for _ in range(2)]
        _STATE["out_bufs"] = bufs
        _STATE["out_idx"] = 0
    buf = bufs[_STATE["out_idx"]]
    _STATE["out_idx"] ^= 1
    np.multiply(yi_h.reshape(B * S, 8, 512), ysc_h[:, :, None],
                out=buf.reshape(B * S, 8, 512))
    return buf.reshape(B, S, D)


def kernel(x, W, bias, lora_a, lora_b, adapter_indices):
    return _run(np.asarray(x), np.asarray(W), np.asarray(bias),
                np.asarray(lora_a), np.asarray(lora_b),
                np.asarray(adapter_indices))


# revision 31
# speedup vs baseline: 1.0285x; 1.0285x over previous
import sys
sys.path.insert(0, '/opt/trn_rl_repo')
import numpy as np
import ml_dtypes
import jax
import jax.numpy as jnp
from jax.sharding import Mesh, PartitionSpec as P, NamedSharding
from jax.experimental.shard_map import shard_map

import concourse.bass as bass
import concourse.mybir as mybir
from concourse.bass2jax import (
    _bass_exec_p, install_neuronx_cc_hook, partition_id_tensor)

# Problem: y[b,s,o] = x[b]@W.T + bias + (x[b]@a[idx[b]].T)@b[idx[b]].T
# B=8 batch elements -> data-parallel, one per NeuronCore.
#
# The axon relay serializes all transfers at ~50MB/s up / ~41MB/s down with
# no up/down overlap, so wall time is dominated by tunnel bytes. The runner
# minimizes them: x uploads int8 (per-512-block abs-max scales) sharded by
# batch, W uploads int8 *sharded* by rows (2MB/core) and is dequantized +
# all-gathered on device, y downloads int8 (per-512-block scales) and is
# dequantized to f32 on the host. Quantization block scales keep the max
# rel err ~8.5e-3 against the f32 reference (gate 2e-2).
B, S, D, RANK = 8, 2048, 4096, 16
P128 = 128
KT = D // P128       # 32 contraction tiles
NQ = 4               # s-quarters
SQ = S // NQ         # 512
NJ = 8               # o-blocks of 512
OJ = D // NJ         # 512
NT = SQ // P128      # 4 s-tiles per quarter
NGROUP = NQ * NJ * NT  # 128 output groups of [128 s, 512 o]

_BF = mybir.dt.bfloat16
_F32 = mybir.dt.float32


def build_nc():
    nc = bass.Bass()
    xt = nc.declare_dram_parameter("xt", [D, S], _BF, isOutput=False)
    wt = nc.declare_dram_parameter("wt", [D, D], _BF, isOutput=False)
    at = nc.declare_dram_parameter("at", [D, RANK], _BF, isOutput=False)
    bt = nc.declare_dram_parameter("bt", [RANK + 1, D], _BF, isOutput=False)
    ones = nc.declare_dram_parameter("ones", [1, S], _BF, isOutput=False)
    y = nc.declare_dram_parameter("y", [S, D], _F32, isOutput=True)

    xt_t = xt.rearrange("(k p) s -> p k s", p=P128)
    wt_t = wt.rearrange("(k p) o -> p k o", p=P128)
    at_t = at.rearrange("(k p) r -> p k r", p=P128)

    with (
        nc.sbuf_tensor([P128, 2, KT, SQ], _BF) as x_sb,
        nc.sbuf_tensor([P128, 2, KT, OJ], _BF) as w_sb,
        nc.sbuf_tensor([P128, KT, RANK], _BF) as at_sb,
        nc.sbuf_tensor([RANK + 1, D], _BF) as bt_sb,
        nc.sbuf_tensor([RANK + 1, S], _BF) as inter_sb,
        nc.sbuf_tensor([P128, 4, OJ], _F32) as out_sb,
        nc.psum_tensor([P128, 7, OJ], _F32) as psum_y,
        nc.psum_tensor([P128, SQ], _F32) as psum_i,
        nc.semaphore("x_sem") as x_sem,
        nc.semaphore("w_sem") as w_sem,
        nc.semaphore("c_sem") as c_sem,
        nc.semaphore("pe_sem") as pe_sem,
        nc.semaphore("pei_sem") as pei_sem,
        nc.semaphore("dve_sem") as dve_sem,
        nc.semaphore("ev_sem") as ev_sem,
        nc.semaphore("st_sem") as st_sem,
        nc.Block() as block,
    ):
        @block.sync
        def _(sync):
            sync.dma_start(at_sb[:], at_t).then_inc(c_sem, 16)
            sync.dma_start(bt_sb[:], bt[:, :]).then_inc(c_sem, 16)
            sync.dma_start(inter_sb[RANK:RANK + 1, :], ones[:, :]).then_inc(c_sem, 16)
            for q in range(NQ):
                if q >= 2:
                    sync.wait_ge(ev_sem, NJ * NT * (q - 1))
                sync.dma_start(
                    x_sb[:, q % 2], xt_t[:, :, q * SQ:(q + 1) * SQ]
                ).then_inc(x_sem, 16)
                for j in range(NJ):
                    wj = q * NJ + j
                    if wj >= 2:
                        sync.wait_ge(ev_sem, NT * (wj - 1))
                    sync.dma_start(
                        w_sb[:, j % 2], wt_t[:, :, j * OJ:(j + 1) * OJ]
                    ).then_inc(w_sem, 16)

        @block.tensor
        def _(tensor):
            tensor.wait_ge(c_sem, 48)
            g = 0
            for q in range(NQ):
                tensor.wait_ge(x_sem, 16 * (q + 1))
                if q > 0:
                    tensor.wait_ge(dve_sem, q)     # psum_i WAR
                for i in range(KT):
                    mm = nc.tensor.matmul(
                        psum_i[0:RANK, :], at_sb[:, i, :], x_sb[:, q % 2, i, :],
                        start=(i == 0), stop=(i == KT - 1),
                    )
                mm.then_inc(pei_sem, 1)
                for j in range(NJ):
                    wj = q * NJ + j
                    tensor.wait_ge(w_sem, 16 * (wj + 1))
                    for t in range(NT):
                        st = q * NT + t
                        if g >= 7:
                            tensor.wait_ge(ev_sem, g - 6)
                        for i in range(KT):
                            nc.tensor.matmul(
                                psum_y[:, g % 7, :],
                                x_sb[:, q % 2, i, t * P128:(t + 1) * P128],
                                w_sb[:, j % 2, i, :],
                                start=(i == 0), stop=False,
                            )
                        tensor.wait_ge(dve_sem, q + 1)
                        nc.tensor.matmul(
                            psum_y[:, g % 7, :],
                            inter_sb[:, st * P128:(st + 1) * P128],
                            bt_sb[:, j * OJ:(j + 1) * OJ],
                            start=False, stop=True,
                        ).then_inc(pe_sem, 1)
                        g += 1

        @block.vector
        def _(vector):
            for q in range(NQ):
                vector.wait_ge(pei_sem, q + 1)
                nc.vector.tensor_copy(
                    inter_sb[0:RANK, q * SQ:(q + 1) * SQ], psum_i[0:RANK, :]
                ).then_inc(dve_sem, 1)

        @block.scalar
        def _(scalar):
            for g in range(NGROUP):
                scalar.wait_ge(pe_sem, g + 1)
                if g >= 4:
                    scalar.wait_ge(st_sem, 16 * (g - 3))
                nc.scalar.copy(out_sb[:, g % 4, :], psum_y[:, g % 7, :]).then_inc(
                    ev_sem, 1
                )

        @block.gpsimd
        def _(gpsimd):
            for g in range(NGROUP):
                q, rem = divmod(g, NJ * NT)
                j, t = divmod(rem, NT)
                st = q * NT + t
                gpsimd.wait_ge(ev_sem, g + 1)
                gpsimd.dma_start(
                    y[st * P128:(st + 1) * P128, j * OJ:(j + 1) * OJ], out_sb[:, g % 4, :]
                ).then_inc(st_sem, 16)

    return nc


_STATE = {}


def _get_state():
    if _STATE:
        return _STATE
    install_neuronx_cc_hook()
    nc = build_nc()
    devs = jax.devices()[:B]
    mesh = Mesh(np.array(devs), ("core",))

    sh_x = NamedSharding(mesh, P("core", None, None))
    sh_w = NamedSharding(mesh, P("core", None))
    sh_r2 = NamedSharding(mesh, P("core", None))

    # stage pre_w: dequantize the row-sharded int8 W, all-gather, transpose.
    # Output is per-core stacked (P("core") over [8*4096, 4096]) so every bass
    # operand uses P("core") like run_bass_via_pjrt does -- replicated
    # operands to the bass custom-call crash the worker.
    def pre_w_body(wq_sh, wsc_sh):              # [512, 4096] i8, [512, 8] f32
        wf = wq_sh.reshape(512, 8, 512).astype(jnp.float32) * wsc_sh[:, :, None]
        wbf = wf.reshape(512, D).astype(jnp.bfloat16)
        w_full = jax.lax.all_gather(wbf, "core", axis=0, tiled=True)  # [4096,4096]
        return w_full.T                          # wt = W.T  [D_in, D_out]

    pre_w = jax.jit(shard_map(
        pre_w_body, mesh=mesh, in_specs=(P("core", None), P("core", None)),
        out_specs=P("core", None), check_rep=False))

    # stage pre_x: per-core dequantize int8 x (per-512-block scales) + transpose
    def pre_x_body(xi_sh, xs_sh):               # [1,2048,4096] i8, [1,2048,8] f32
        xf = xi_sh[0].reshape(S, 8, 512).astype(jnp.float32) * xs_sh[0][:, :, None]
        return xf.reshape(S, D).astype(jnp.bfloat16).T   # [4096, 2048]

    pre_x = jax.jit(shard_map(
        pre_x_body, mesh=mesh,
        in_specs=(P("core", None, None), P("core", None, None)),
        out_specs=P("core", None), check_rep=False))

    # bass stage: the hand-written kernel, one batch element per core.
    # The trailing partition_id operand is mandatory: bass.Bass() declares a
    # partition_id ExternalInput, and leaving it unbound crashes the worker.
    out_avals = (jax.core.ShapedArray((S, D), np.float32),)

    def bass_body(xt, wt, at, bt, ones):
        outs = _bass_exec_p.bind(
            xt, wt, at, bt, ones, partition_id_tensor(),
            out_avals=out_avals,
            in_names=("xt", "wt", "at", "bt", "ones", "partition_id"),
            out_names=("y",),
            lowering_input_output_aliases=(),
            sim_require_finite=True,
            sim_require_nnan=True,
            nc=nc,
        )
        return outs[0]

    bass_sm = jax.jit(shard_map(
        bass_body, mesh=mesh,
        in_specs=(P("core", None), P("core", None), P("core", None),
                  P("core", None), P("core")),
        out_specs=P("core", None), check_rep=False), keep_unused=True)

    # stage post: quantize y to int8 with per-512-block scales for the
    # download, split into two halves so the host can dequantize half 0
    # while half 1 is still streaming down.
    H = S // 2

    def post_body(y):                           # [2048, 4096] f32 per core
        yr = y.reshape(S, 8, 512)
        m = jnp.max(jnp.abs(yr), axis=2, keepdims=True)
        scale = jnp.maximum(m, 1e-30) * (1.0 / 127.0)
        yi = jnp.round(yr / scale).astype(jnp.int8).reshape(S, D)
        sc = scale[:, :, 0]
        return yi[:H], yi[H:], sc[:H], sc[H:]

    post = jax.jit(shard_map(
        post_body, mesh=mesh, in_specs=P("core", None),
        out_specs=(P("core", None),) * 4, check_rep=False))

    _STATE.update(dict(nc=nc, mesh=mesh, sh_x=sh_x, sh_w=sh_w, sh_r2=sh_r2,
                       pre_w=pre_w, pre_x=pre_x, bass_sm=bass_sm, post=post))
    return _STATE


def _run(x, W, bias, lora_a, lora_b, adapter_indices):
    st = _get_state()
    mesh = st["mesh"]

    # small tables on host (bf16, a few MB total)
    idx = [int(i) for i in np.asarray(adapter_indices)]
    mw = np.abs(W).reshape(D, 8, 512).max(axis=-1)              # [4096, 8]
    wsc = (np.maximum(mw, 1e-30) * np.float32(1.0 / 127.0)).astype(np.float32)
    wq = np.rint(W.reshape(D, 8, 512) * (np.float32(1.0) / wsc)[:, :, None]
                 ).astype(np.int8).reshape(D, D)
    at_g = np.concatenate(
        [lora_a[i].T for i in idx], axis=0).astype(ml_dtypes.bfloat16)   # [8*4096, 16]
    bias32 = bias.astype(np.float32)
    bt_g = np.concatenate(
        [np.concatenate([lora_b[i].astype(np.float32).T, bias32[None, :]], axis=0)
         for i in idx], axis=0).astype(ml_dtypes.bfloat16)      # [8*17, 4096]
    ones_g = np.ones((B, S), dtype=ml_dtypes.bfloat16)

    # dispatch W + tables first so the relay streams them while we quantize x
    wd = jax.device_put(wq, st["sh_w"])
    wscd = jax.device_put(wsc, NamedSharding(mesh, P("core")))
    atd = jax.device_put(at_g, st["sh_r2"])
    btd = jax.device_put(bt_g, st["sh_r2"])
    onesd = jax.device_put(ones_g, NamedSharding(mesh, P("core")))
    wtd = st["pre_w"](wd, wscd)

    # quantize x to int8 with per-row abs-max scales (halves the upload).
    # Fused passes with preallocated buffers; round-half-up via the
    # +128.5-truncate-xor trick (uint8 cast truncates, values positive).
    tmp = _STATE.get("tmp_f32")
    if tmp is None:
        tmp = np.empty((B, S, D), np.float32)
        _STATE["tmp_f32"] = tmp
        _STATE["u8"] = np.empty((B, S, D), np.uint8)
        _STATE["mxb"] = np.empty((B, S, 8), np.float32)
    u8 = _STATE["u8"]
    np.abs(x, out=tmp)
    mx = np.max(tmp.reshape(B, S, 8, 512), axis=-1, out=_STATE["mxb"])
    inv = np.float32(127.0) / np.maximum(mx, 1e-30, out=mx)
    np.multiply(x.reshape(B, S, 8, 512), inv[:, :, :, None],
                out=tmp.reshape(B, S, 8, 512))
    tmp += np.float32(128.5)
    np.copyto(u8, tmp, casting='unsafe')
    np.bitwise_xor(u8, 0x80, out=u8)
    xi8 = u8.view(np.int8)
    xs = (np.float32(1.0) / inv).astype(np.float32)                # [8,2048,8]
    xd = jax.device_put(xi8, st["sh_x"])
    xsd = jax.device_put(xs, NamedSharding(mesh, P("core", None, None)))
    xtd = st["pre_x"](xd, xsd)

    yd = st["bass_sm"](xtd, wtd, atd, btd, onesd)
    yi0, yi1, ysc0, ysc1 = st["post"](yd)
    for a in (yi0, ysc0, yi1, ysc1):
        try:
            a.copy_to_host_async()
        except Exception:
            pass
    # double-buffered output so two successive calls don't alias one array
    bufs = _STATE.get("out_bufs")
    if bufs is None:
        bufs = [np.empty((B, S, 8, 512), dtype=np.float32) for _ in range(2)]
        _STATE["out_bufs"] = bufs
        _STATE["out_idx"] = 0
    buf = bufs[_STATE["out_idx"]]
    _STATE["out_idx"] ^= 1
    H = S // 2
    # half 0 arrives first; dequantize it while half 1 still streams
    h0, s0 = jax.device_get((yi0, ysc0))        # [8*H,4096] i8, [8*H,8] f32
    np.multiply(h0.reshape(B, H, 8, 512), s0.reshape(B, H, 8)[:, :, :, None],
                out=buf[:, :H])
    h1, s1 = jax.device_get((yi1, ysc1))
    np.multiply(h1.reshape(B, H, 8, 512), s1.reshape(B, H, 8)[:, :, :, None],
                out=buf[:, H:])
    # free device buffers eagerly so they don't pile up on the terminal
    for a in (xd, xsd, xtd, wd, wscd, wtd, atd, btd, onesd, yd,
              yi0, yi1, ysc0, ysc1):
        try:
            a.delete()
        except Exception:
            pass
    return buf.reshape(B, S, D)


def kernel(x, W, bias, lora_a, lora_b, adapter_indices):
    return _run(np.asarray(x), np.asarray(W), np.asarray(bias),
                np.asarray(lora_a), np.asarray(lora_b),
                np.asarray(adapter_indices))
